# revision 16
# baseline (speedup 1.0000x reference)
"""Trainium2 Bass kernel for nn_Block_82111184765408, v3.

f32r datapath (bf16/fp8 matmuls pay an un-hidden LdWeights penalty on HW;
f32r at N>=256 runs at 1 cycle/row with internal weight load). Structural
improvements over the baseline:
  - software-pipelined pair loop: pair N+1's x-DMA prefetch and LN1+QKV are
    emitted before pair N's LN2+FFN, so the PE is fed during the serial
    LN DVE chains.
  - per-stage PSUM pools (scores/AV/rps/transpose/matmul) so the attention
    head loop pipelines instead of serializing on one rotating pool.
  - LN rstd via bit-trick + Newton on DVE: no ACT Sqrt => no activation
    table reloads (exp/relu/copy/identity share one table set).
  - LN second moment via scalar_tensor_tensor with accum_out (no separate
    ACT square pass).
  - f32r transposes (1.5 cycles/row vs 2.0 for f32).
  - x DMA split across three queues; V-hat width 65 (no junk rows).
  - engine rebalance: causal-mask multiplies on Pool (SBUF-only, otherwise
    idle), LN gain/bias fold fused into the ACT psum->SBUF copy via
    per-partition scale+bias APs; DVE keeps stats/recip/residuals.
"""

import contextlib

import numpy as np

import concourse.bass as bass
import concourse.mybir as mybir
import concourse.tile as tile
from concourse import bacc
from concourse.bass_utils import run_bass_kernel_spmd
from concourse.masks import make_identity

P = 128
B, T, C, H, D = 128, 256, 384, 6, 64
FF = 4 * C
N_CORES = 8
B_LOCAL = B // N_CORES
N_PAIRS = B_LOCAL // 2
TP = 2 * T
CC = C // P
FC = FF // P
EPS = 1e-5
SCALE = C ** -0.5
MAGIC = 0x5F3759DF

f32 = mybir.dt.float32
f32r = mybir.dt.float32r
i32 = mybir.dt.int32
AF = mybir.ActivationFunctionType
OP = mybir.AluOpType


def build_nc(n_pairs=N_PAIRS, debug_outputs=False, repeat=1):
    nc = bacc.Bacc("TRN2", target_bir_lowering=False, debug=False)

    x_d = nc.declare_dram_parameter("x", [2 * n_pairs, T, C], f32, isOutput=False)
    ln1_g_d = nc.declare_dram_parameter("ln1_g", [C], f32, isOutput=False)
    ln1_b_d = nc.declare_dram_parameter("ln1_b", [C], f32, isOutput=False)
    Wk_d = nc.declare_dram_parameter("Wk", [H, C, D], f32r, isOutput=False)
    bk_d = nc.declare_dram_parameter("bk", [H, D], f32, isOutput=False)
    Wq_d = nc.declare_dram_parameter("Wq", [H, C, D], f32r, isOutput=False)
    bq_d = nc.declare_dram_parameter("bq", [H, D], f32, isOutput=False)
    Wv_d = nc.declare_dram_parameter("Wv", [H, C, D], f32r, isOutput=False)
    bv_d = nc.declare_dram_parameter("bv", [H, D], f32r, isOutput=False)
    Wp_d = nc.declare_dram_parameter("Wp", [C, C], f32r, isOutput=False)
    bp_d = nc.declare_dram_parameter("bp", [C], f32, isOutput=False)
    ln2_g_d = nc.declare_dram_parameter("ln2_g", [C], f32, isOutput=False)
    ln2_b_d = nc.declare_dram_parameter("ln2_b", [C], f32, isOutput=False)
    W1_d = nc.declare_dram_parameter("W1", [C, FF], f32r, isOutput=False)
    b1_d = nc.declare_dram_parameter("b1", [FF], f32, isOutput=False)
    W2_d = nc.declare_dram_parameter("W2", [FF, C], f32r, isOutput=False)
    b2_d = nc.declare_dram_parameter("b2", [C], f32, isOutput=False)
    y_d = nc.declare_dram_parameter("y", [2 * n_pairs, T, C], f32, isOutput=True)

    with tile.TileContext(nc) as tc:
        with tc.tile_pool(name="const", bufs=1) as cst, \
             tc.tile_pool(name="ps", bufs=2, space="PSUM") as psp, \
             tc.tile_pool(name="sc", bufs=2, space="PSUM") as scp, \
             tc.tile_pool(name="av", bufs=2, space="PSUM") as avp, \
             tc.tile_pool(name="pt", bufs=2, space="PSUM") as ptp:

            def psum():
                return psp.tile([P, TP], f32, tag="ps", name="ps")

            def psum_sc():
                return scp.tile([P, TP], f32, tag="sc", name="sc")

            def psum_av():
                return avp.tile([P, TP], f32, tag="av", name="av")

            psum_rp = psum

            def psum_t():
                return ptp.tile([P, TP], f32r, tag="pt", name="pt")

            # ---------- constants ----------
            Wq_sb = cst.tile([P, CC, C], f32r, tag="Wq")
            for h in range(H):
                nc.sync.dma_start(Wq_sb[:, :, 64 * h:64 * h + 64],
                                  Wq_d[h].rearrange("(o p) d -> p o d", p=P))
            Wk_sb = cst.tile([P, CC, C], f32r, tag="Wk")
            for h in range(H):
                nc.sync.dma_start(Wk_sb[:, :, 64 * h:64 * h + 64],
                                  Wk_d[h].rearrange("(o p) d -> p o d", p=P))
            Wv_sb = cst.tile([P, CC, C], f32r, tag="Wv")
            for h in range(H):
                nc.sync.dma_start(Wv_sb[:, :, 64 * h:64 * h + 64],
                                  Wv_d[h].rearrange("(o p) d -> p o d", p=P))
            Wp_sb = cst.tile([P, CC, C], f32r, tag="Wp")
            nc.sync.dma_start(Wp_sb[:], Wp_d.rearrange("(o p) c -> p o c", p=P))
            W1_sb = cst.tile([P, CC, FF], f32r, tag="W1")
            nc.sync.dma_start(W1_sb[:], W1_d.rearrange("(o p) f -> p o f", p=P))
            W2_sb = cst.tile([P, FC, C], f32r, tag="W2")
            nc.sync.dma_start(W2_sb[:], W2_d.rearrange("(o p) c -> p o c", p=P))

            g1_sb = cst.tile([P, CC], f32, tag="g1")
            nc.sync.dma_start(g1_sb[:], ln1_g_d.rearrange("(o p) -> p o", p=P))
            lb1_sb = cst.tile([P, CC], f32, tag="lb1")
            nc.sync.dma_start(lb1_sb[:], ln1_b_d.rearrange("(o p) -> p o", p=P))
            g2_sb = cst.tile([P, CC], f32, tag="g2")
            nc.sync.dma_start(g2_sb[:], ln2_g_d.rearrange("(o p) -> p o", p=P))
            lb2_sb = cst.tile([P, CC], f32, tag="lb2")
            nc.sync.dma_start(lb2_sb[:], ln2_b_d.rearrange("(o p) -> p o", p=P))

            bq_sb = cst.tile([P, CC], f32, tag="bq")
            nc.sync.dma_start(
                bq_sb[:], bq_d.rearrange("h d -> (h d)").rearrange("(o p) -> p o", p=P))
            bk_sb = cst.tile([P, CC], f32, tag="bk")
            nc.sync.dma_start(
                bk_sb[:], bk_d.rearrange("h d -> (h d)").rearrange("(o p) -> p o", p=P))
            bvrow = cst.tile([1, C], f32r, tag="bvrow")
            nc.sync.dma_start(bvrow[:], bv_d.rearrange("h d -> (h d)")[None])
            bp_sb = cst.tile([P, CC], f32, tag="bp")
            nc.sync.dma_start(bp_sb[:], bp_d.rearrange("(o p) -> p o", p=P))
            b1f_sb = cst.tile([P, FC], f32, tag="b1f")
            nc.sync.dma_start(b1f_sb[:], b1_d.rearrange("(o p) -> p o", p=P))
            b2_sb = cst.tile([P, CC], f32, tag="b2")
            nc.sync.dma_start(b2_sb[:], b2_d.rearrange("(o p) -> p o", p=P))

            ident = cst.tile([P, P], f32, tag="ident")
            make_identity(nc, ident[:])
            identr_t = cst.tile([P, P], f32r, tag="identr")
            with nc.allow_low_precision(reason="f32r identity"):
                nc.vector.tensor_scalar(identr_t[:], ident[:], 1.0, None, OP.mult)
            identr = identr_t[:]

            ones64 = cst.tile([P, P], f32r, tag="ones64")
            nc.vector.tensor_scalar(ones64[64:65, :], ident[64:65, :], 0.0, 1.0,
                                    OP.mult, OP.add)

            ones_col = cst.tile([1, P], f32r, tag="ones_col")
            nc.vector.tensor_scalar(ones_col[:], ident[0:1, :], 0.0, 1.0,
                                    OP.mult, OP.add)

            # causal masks, multiplicative; each [P, 256]
            # mask_sc[p, tl] = 1 if (p + 128*sc) <= tl else 0
            masks = []
            for sc in range(2):
                mk = cst.tile([P, 256], f32, tag=f"mask{sc}")
                nc.gpsimd.memset(mk[:], 1.0)
                nc.gpsimd.affine_select(
                    out=mk[:], in_=mk[:],
                    compare_op=OP.is_ge, fill=0.0,
                    base=-128 * sc, pattern=[[1, 256]], channel_multiplier=-1)
                masks.append(mk)

            # ---------- per-pair loop (software-pipelined) ----------
            rep_ctx = tc.For_i(0, repeat, 1) if repeat > 1 else contextlib.nullcontext()
            with tc.tile_pool(name="p2", bufs=2) as p2, \
                 tc.tile_pool(name="p1", bufs=1) as p1, \
                 rep_ctx:

                  def stage_dma(pr):
                      x_view = x_d[2 * pr:2 * pr + 2].rearrange(
                          "b (o p) c -> p (b o) c", p=P)
                      x_tok = p2.tile([P, 4, C], f32, tag="x_tok")
                      nc.sync.dma_start(x_tok[:, 0:2], x_view[:, 0:2])
                      nc.scalar.dma_start(x_tok[:, 2:3], x_view[:, 2:3])
                      nc.gpsimd.dma_start(x_tok[:, 3:4], x_view[:, 3:4])
                      return x_tok

                  def layernorm_tokens(src, g_sb, lb_sb, dstT, tagp,
                                       act_copy=True, s_pre=None):
                      """src: [P,4,C] token-major f32. Writes dstT [P,CC,TP]
                      f32r feature-major, gain/bias folded into the copy.
                      s_pre: optional precomputed per-slot row sums [P,4]."""
                      if s_pre is None:
                          s = p1.tile([P, 4], f32, tag=f"{tagp}_s")
                          nc.vector.tensor_reduce(s[:], src[:],
                                                  mybir.AxisListType.X, OP.add)
                      else:
                          s = s_pre
                      sq = p1.tile([P, C], f32, tag="sq_scratch")
                      s2 = p1.tile([P, 4], f32, tag=f"{tagp}_s2")
                      for so in range(4):
                          nc.vector.scalar_tensor_tensor(
                              sq[:], src[:, so], 1.0, src[:, so],
                              OP.mult, OP.mult, accum_out=s2[:, so:so + 1])
                      mu = p1.tile([P, 4], f32, tag=f"{tagp}_mu")
                      nc.vector.tensor_scalar_mul(mu[:], s[:], 1.0 / C)
                      var = p1.tile([P, 4], f32, tag=f"{tagp}_var")
                      nc.vector.tensor_scalar(var[:], s2[:], 1.0 / C, EPS,
                                              OP.mult, OP.add)
                      mu2 = p1.tile([P, 4], f32, tag=f"{tagp}_mu2")
                      nc.vector.tensor_tensor(mu2[:], mu[:], mu[:], OP.mult)
                      nc.vector.tensor_tensor(var[:], var[:], mu2[:], OP.subtract)
                      # rstd via bit-trick + Newton (no ACT table switch)
                      yt = p1.tile([P, 4], f32, tag=f"{tagp}_y")
                      ti = p1.tile([P, 4], i32, tag=f"{tagp}_ti")
                      nc.vector.tensor_scalar(ti[:], var[:].bitcast(i32), 1, None,
                                              OP.arith_shift_right)
                      nc.vector.tensor_scalar(yt[:].bitcast(i32), ti[:], -1, MAGIC,
                                              OP.mult, OP.add)
                      y2 = p1.tile([P, 4], f32, tag=f"{tagp}_y2")
                      w = p1.tile([P, 4], f32, tag=f"{tagp}_w")
                      for _ in range(2):
                          nc.vector.tensor_tensor(y2[:], yt[:], yt[:], OP.mult)
                          nc.vector.tensor_tensor(y2[:], y2[:], var[:], OP.mult)
                          nc.vector.tensor_scalar(w[:], y2[:], -0.5, 1.5,
                                                  OP.mult, OP.add)
                          nc.vector.tensor_tensor(yt[:], yt[:], w[:], OP.mult)
                      murs = p1.tile([P, 4], f32, tag=f"{tagp}_murs")
                      nc.vector.tensor_tensor(murs[:], mu[:], yt[:], OP.mult)
                      htok = p1.tile([P, 4, C], f32r, tag="htok", name="htok")
                      with nc.allow_low_precision(reason="f32r htok"):
                          for so in range(4):
                              nc.vector.tensor_scalar(
                                  htok[:, so], src[:, so],
                                  yt[:, so:so + 1], murs[:, so:so + 1],
                                  OP.mult, OP.subtract)
                      for c in range(CC):
                          tp = psum_t()
                          for so in range(4):
                              nc.tensor.transpose(
                                  tp[:, P * so:P * so + P],
                                  htok[:, so, P * c:P * c + P], identr)
                          if act_copy:
                              nc.scalar.activation(
                                  dstT[:, c], tp[:].bitcast(f32), AF.Identity,
                                  bias=lb_sb[:, c:c + 1], scale=g_sb[:, c:c + 1])
                          else:
                              nc.vector.tensor_scalar(
                                  dstT[:, c], tp[:].bitcast(f32),
                                  g_sb[:, c:c + 1], lb_sb[:, c:c + 1],
                                  OP.mult, OP.add)

                  def stage_qkv(x_tok):
                      h1T = p2.tile([P, CC, TP], f32r, tag="h1T")
                      layernorm_tokens(x_tok, g1_sb, lb1_sb, h1T, "ln1")
                      QT = p2.tile([P, CC, TP], f32r, tag="QT")
                      KT = p2.tile([P, CC, TP], f32r, tag="KT")
                      for (W_sb, b_sb, dst) in ((Wq_sb, bq_sb, QT),
                                                (Wk_sb, bk_sb, KT)):
                          for mo in range(CC):
                              ps = psum()
                              for c in range(CC):
                                  nc.tensor.matmul(
                                      ps[:], W_sb[:, c, P * mo:P * mo + P],
                                      h1T[:, c],
                                      start=(c == 0), stop=(c == CC - 1))
                              nc.scalar.activation(dst[:, mo], ps[:], AF.Identity,
                                                   bias=b_sb[:, mo:mo + 1])
                      # V-hat [P, 4, H, 65]: col 64 = 1.0 (denominator row)
                      V_sb = p2.tile([P, 4, H, 65], f32r, tag="V_sb")
                      with nc.allow_low_precision(reason="ones col"):
                          nc.vector.tensor_scalar(
                              V_sb[:, :, :, 64:65],
                              ident[:, 0:1][:, None, None, :]
                              .to_broadcast((P, 4, H, 1)),
                              0.0, 1.0, OP.mult, OP.add)
                      for to in range(4):
                          ps = psum()
                          for c in range(CC):
                              nc.tensor.matmul(
                                  ps[:, 0:C], h1T[:, c, P * to:P * to + P],
                                  Wv_sb[:, c],
                                  start=(c == 0), stop=False)
                          nc.tensor.matmul(ps[:, 0:C], ones_col[:], bvrow[:],
                                           start=False, stop=True)
                          nc.scalar.activation(
                              V_sb[:, to, :, 0:64],
                              ps[:, 0:C].rearrange("p (h d) -> p h d", h=H),
                              AF.Copy)
                      return QT, KT, V_sb

                  x_cur = stage_dma(0)
                  qkv_cur = stage_qkv(x_cur)
                  for pr in range(n_pairs):
                   x_tok = x_cur
                   QT, KT, V_sb = qkv_cur
                   y_view = y_d[2 * pr:2 * pr + 2].rearrange(
                       "b (o p) c -> p (b o) c", p=P)
                   if pr + 1 < n_pairs:
                       x_cur = stage_dma(pr + 1)

                   # ---- attention ----
                   AVT = p1.tile([P, CC, TP], f32r, tag="AVT", name="AVT")
                   for h in range(H):
                       mo, half = h // 2, h % 2
                       rows = slice(64 * half, 64 * half + 64)
                       Es = []
                       tri = masks[0][:, 0:128]
                       for sc in range(2):
                           sps = psum_sc()
                           for bb in range(2):
                               cols = slice(256 * bb, 256 * bb + 256)
                               nc.tensor.matmul(
                                   sps[:, cols],
                                   QT[rows, mo, 256 * bb + 128 * sc:
                                      256 * bb + 128 * sc + 128],
                                   KT[rows, mo, cols],
                                   start=True, stop=True)
                           e = p2.tile([P, TP], f32r, tag=f"E{sc}")
                           ev = e[:].rearrange("p (b t) -> p b t", b=2)
                           evf = e[:].bitcast(f32).rearrange(
                               "p (b t) -> p b t", b=2)
                           spsv = sps[:].rearrange("p (b t) -> p b t", b=2)
                           if sc == 0:
                               # lower t-halves need the triangle; upper
                               # halves are all-keep (p < 128 <= t)
                               nc.scalar.activation(e[:], sps[:], AF.Exp,
                                                    scale=SCALE)
                               nc.gpsimd.tensor_tensor(
                                   ev[:, :, 0:128], evf[:, :, 0:128],
                                   tri[:, None, :].to_broadcast((P, 2, P)),
                                   OP.mult)
                           else:
                               # lower t-halves are fully masked: write 0,
                               # exp only the upper halves + triangle
                               with nc.allow_low_precision(reason="zero f32r"):
                                   nc.gpsimd.tensor_scalar(
                                       ev[:, :, 0:128],
                                       tri[:, None, :].to_broadcast((P, 2, P)),
                                       0.0, None, OP.mult)
                               nc.scalar.activation(
                                   ev[:, :, 128:256], spsv[:, :, 128:256],
                                   AF.Exp, scale=SCALE)
                               nc.gpsimd.tensor_tensor(
                                   ev[:, :, 128:256], evf[:, :, 128:256],
                                   tri[:, None, :].to_broadcast((P, 2, P)),
                                   OP.mult)
                           Es.append(e)
                       avps = psum_av()
                       for bb in range(2):
                           cols = slice(256 * bb, 256 * bb + 256)
                           for sc in range(2):
                               nc.tensor.matmul(
                                   avps[0:65, cols], V_sb[:, 2 * bb + sc, h, :],
                                   Es[sc][:, cols], start=(sc == 0),
                                   stop=(sc == 1))
                       R = p2.tile([P, TP], f32r, tag="R", name="R")
                       with nc.allow_low_precision(reason="softmax denom recip"):
                           nc.vector.reciprocal(R[64:65, :], avps[64:65, :])
                       rps = psum_rp()
                       nc.tensor.matmul(rps[:], ones64[64:65, :], R[64:65, :],
                                        start=True, stop=True)
                       nc.scalar.activation(AVT[rows, mo, :], avps[0:64, :],
                                            AF.Copy)
                       nc.vector.tensor_tensor(AVT[rows, mo],
                                               AVT[rows, mo].bitcast(f32),
                                               rps[rows, :], OP.mult)

                   # ---- proj + residual ----
                   proj_sb = p1.tile([P, CC, TP], f32r, tag="proj_sb")
                   for mo in range(CC):
                       ps = psum()
                       for c in range(CC):
                           nc.tensor.matmul(
                               ps[:], Wp_sb[:, c, P * mo:P * mo + P], AVT[:, c],
                               start=(c == 0), stop=(c == CC - 1))
                       nc.scalar.activation(proj_sb[:, mo], ps[:], AF.Identity,
                                            bias=bp_sb[:, mo:mo + 1])
                   out1_tok = p1.tile([P, 4, C], f32, tag="out1_tok")
                   ln2_s = p1.tile([P, 4], f32, tag="ln2_s")
                   for so in range(4):
                       tp = psum_t()
                       for mo in range(CC):
                           nc.tensor.transpose(
                               tp[:, P * mo:P * mo + P],
                               proj_sb[:, mo, P * so:P * so + P], identr)
                       nc.vector.scalar_tensor_tensor(
                           out1_tok[:, so], tp[:, 0:C].bitcast(f32), 1.0,
                           x_tok[:, so], OP.mult, OP.add,
                           accum_out=ln2_s[:, so:so + 1])

                   # ---- next pair's LN1 + QKV (fills PE during LN2) ----
                   if pr + 1 < n_pairs:
                       qkv_cur = stage_qkv(x_cur)

                   # ---- LN2 + FFN ----
                   h2T = p1.tile([P, CC, TP], f32r, tag="h2T")
                   layernorm_tokens(out1_tok, g2_sb, lb2_sb, h2T, "ln2", s_pre=ln2_s)

                   FF_sb = p1.tile([P, FC, TP], f32r, tag="FF_sb")
                   for fo in range(FC):
                       ps = psum()
                       for c in range(CC):
                           nc.tensor.matmul(
                               ps[:], W1_sb[:, c, P * fo:P * fo + P], h2T[:, c],
                               start=(c == 0), stop=(c == CC - 1))
                       nc.scalar.activation(FF_sb[:, fo], ps[:], AF.Relu,
                                            bias=b1f_sb[:, fo:fo + 1])
                   g_sb = p1.tile([P, CC, TP], f32r, tag="g_sb")
                   for mo in range(CC):
                       ps = psum()
                       for fo in range(FC):
                           nc.tensor.matmul(
                               ps[:], W2_sb[:, fo, P * mo:P * mo + P], FF_sb[:, fo],
                               start=(fo == 0), stop=(fo == FC - 1))
                       nc.scalar.activation(g_sb[:, mo], ps[:], AF.Identity,
                                            bias=b2_sb[:, mo:mo + 1])

                   y_tok = p2.tile([P, 4, C], f32, tag="y_tok")
                   for so in range(4):
                       tp = psum_t()
                       for mo in range(CC):
                           nc.tensor.transpose(
                               tp[:, P * mo:P * mo + P],
                               g_sb[:, mo, P * so:P * so + P], identr)
                       nc.vector.tensor_tensor(y_tok[:, so],
                                               tp[:, 0:C].bitcast(f32),
                                               out1_tok[:, so], OP.add)
                   nc.sync.dma_start(y_view, y_tok[:])

    nc.compile()
    return nc


_NC_CACHE = {}


def kernel(_run_kwargs=None, **inputs) -> np.ndarray:
    run_kwargs = _run_kwargs or {}
    x = np.ascontiguousarray(np.asarray(inputs["x"], dtype=np.float32))
    weights = {k: np.ascontiguousarray(np.asarray(v, dtype=np.float32))
               for k, v in inputs.items() if k != "x"}

    if "nc" not in _NC_CACHE:
        _NC_CACHE["nc"] = build_nc()
    nc = _NC_CACHE["nc"]

    in_maps = []
    for c in range(N_CORES):
        m = {"x": x[c * B_LOCAL:(c + 1) * B_LOCAL]}
        m.update(weights)
        in_maps.append(m)

    res = run_bass_kernel_spmd(nc, in_maps, core_ids=list(range(N_CORES)), **run_kwargs)
    y = np.concatenate([r["y"] for r in res.results], axis=0)
    kernel.last_result = res
    return y


# revision 18
# speedup vs baseline: 2.2163x; 2.2163x over previous
"""Trainium2 Bass kernel for nn_Block_82111184765408, v3.

f32r datapath (bf16/fp8 matmuls pay an un-hidden LdWeights penalty on HW;
f32r at N>=256 runs at 1 cycle/row with internal weight load). Structural
improvements over the baseline:
  - software-pipelined pair loop: pair N+1's x-DMA prefetch and LN1+QKV are
    emitted before pair N's LN2+FFN, so the PE is fed during the serial
    LN DVE chains.
  - per-stage PSUM pools (scores/AV/rps/transpose/matmul) so the attention
    head loop pipelines instead of serializing on one rotating pool.
  - LN rstd via bit-trick + Newton on DVE: no ACT Sqrt => no activation
    table reloads (exp/relu/copy/identity share one table set).
  - LN second moment via scalar_tensor_tensor with accum_out (no separate
    ACT square pass).
  - f32r transposes (1.5 cycles/row vs 2.0 for f32).
  - x DMA split across three queues; V-hat width 65 (no junk rows).
  - engine rebalance: causal-mask multiplies on Pool (SBUF-only, otherwise
    idle), LN gain/bias fold fused into the ACT psum->SBUF copy via
    per-partition scale+bias APs; DVE keeps stats/recip/residuals.
"""

import contextlib

import numpy as np

import concourse.bass as bass
import concourse.mybir as mybir
import concourse.tile as tile
from concourse import bacc
from concourse.bass_utils import run_bass_kernel_spmd
from concourse.masks import make_identity

P = 128
B, T, C, H, D = 128, 256, 384, 6, 64
FF = 4 * C
N_CORES = 8
B_LOCAL = B // N_CORES
N_PAIRS = B_LOCAL // 2
TP = 2 * T
CC = C // P
FC = FF // P
EPS = 1e-5
SCALE = C ** -0.5
MAGIC = 0x5F3759DF

f32 = mybir.dt.float32
f32r = mybir.dt.float32r
i32 = mybir.dt.int32
AF = mybir.ActivationFunctionType
OP = mybir.AluOpType


def build_nc(n_pairs=N_PAIRS, debug_outputs=False, repeat=1):
    nc = bacc.Bacc("TRN2", target_bir_lowering=False, debug=False)

    x_d = nc.declare_dram_parameter("x", [2 * n_pairs, T, C], f32, isOutput=False)
    ln1_g_d = nc.declare_dram_parameter("ln1_g", [C], f32, isOutput=False)
    ln1_b_d = nc.declare_dram_parameter("ln1_b", [C], f32, isOutput=False)
    Wk_d = nc.declare_dram_parameter("Wk", [H, C, D], f32r, isOutput=False)
    bk_d = nc.declare_dram_parameter("bk", [H, D], f32, isOutput=False)
    Wq_d = nc.declare_dram_parameter("Wq", [H, C, D], f32r, isOutput=False)
    bq_d = nc.declare_dram_parameter("bq", [H, D], f32, isOutput=False)
    Wv_d = nc.declare_dram_parameter("Wv", [H, C, D], f32r, isOutput=False)
    bv_d = nc.declare_dram_parameter("bv", [H, D], f32r, isOutput=False)
    Wp_d = nc.declare_dram_parameter("Wp", [C, C], f32r, isOutput=False)
    bp_d = nc.declare_dram_parameter("bp", [C], f32, isOutput=False)
    ln2_g_d = nc.declare_dram_parameter("ln2_g", [C], f32, isOutput=False)
    ln2_b_d = nc.declare_dram_parameter("ln2_b", [C], f32, isOutput=False)
    W1_d = nc.declare_dram_parameter("W1", [C, FF], f32r, isOutput=False)
    b1_d = nc.declare_dram_parameter("b1", [FF], f32, isOutput=False)
    W2_d = nc.declare_dram_parameter("W2", [FF, C], f32r, isOutput=False)
    b2_d = nc.declare_dram_parameter("b2", [C], f32, isOutput=False)
    y_d = nc.declare_dram_parameter("y", [2 * n_pairs, T, C], f32, isOutput=True)

    with tile.TileContext(nc) as tc:
        with tc.tile_pool(name="const", bufs=1) as cst, \
             tc.tile_pool(name="ps", bufs=2, space="PSUM") as psp, \
             tc.tile_pool(name="sc", bufs=2, space="PSUM") as scp, \
             tc.tile_pool(name="av", bufs=2, space="PSUM") as avp, \
             tc.tile_pool(name="pt", bufs=2, space="PSUM") as ptp:

            def psum():
                return psp.tile([P, TP], f32, tag="ps", name="ps")

            def psum_sc():
                return scp.tile([P, TP], f32, tag="sc", name="sc")

            def psum_av():
                return avp.tile([P, TP], f32, tag="av", name="av")

            psum_rp = psum

            def psum_t():
                return ptp.tile([P, TP], f32r, tag="pt", name="pt")

            # ---------- constants ----------
            Wq_sb = cst.tile([P, CC, C], f32r, tag="Wq")
            for h in range(H):
                nc.sync.dma_start(Wq_sb[:, :, 64 * h:64 * h + 64],
                                  Wq_d[h].rearrange("(o p) d -> p o d", p=P))
            Wk_sb = cst.tile([P, CC, C], f32r, tag="Wk")
            for h in range(H):
                nc.sync.dma_start(Wk_sb[:, :, 64 * h:64 * h + 64],
                                  Wk_d[h].rearrange("(o p) d -> p o d", p=P))
            Wv_sb = cst.tile([P, CC, C], f32r, tag="Wv")
            for h in range(H):
                nc.sync.dma_start(Wv_sb[:, :, 64 * h:64 * h + 64],
                                  Wv_d[h].rearrange("(o p) d -> p o d", p=P))
            Wp_sb = cst.tile([P, CC, C], f32r, tag="Wp")
            nc.sync.dma_start(Wp_sb[:], Wp_d.rearrange("(o p) c -> p o c", p=P))
            W1_sb = cst.tile([P, CC, FF], f32r, tag="W1")
            nc.sync.dma_start(W1_sb[:], W1_d.rearrange("(o p) f -> p o f", p=P))
            W2_sb = cst.tile([P, FC, C], f32r, tag="W2")
            nc.sync.dma_start(W2_sb[:], W2_d.rearrange("(o p) c -> p o c", p=P))

            g1_sb = cst.tile([P, CC], f32, tag="g1")
            nc.sync.dma_start(g1_sb[:], ln1_g_d.rearrange("(o p) -> p o", p=P))
            lb1_sb = cst.tile([P, CC], f32, tag="lb1")
            nc.sync.dma_start(lb1_sb[:], ln1_b_d.rearrange("(o p) -> p o", p=P))
            g2_sb = cst.tile([P, CC], f32, tag="g2")
            nc.sync.dma_start(g2_sb[:], ln2_g_d.rearrange("(o p) -> p o", p=P))
            lb2_sb = cst.tile([P, CC], f32, tag="lb2")
            nc.sync.dma_start(lb2_sb[:], ln2_b_d.rearrange("(o p) -> p o", p=P))

            bq_sb = cst.tile([P, CC], f32, tag="bq")
            nc.sync.dma_start(
                bq_sb[:], bq_d.rearrange("h d -> (h d)").rearrange("(o p) -> p o", p=P))
            bk_sb = cst.tile([P, CC], f32, tag="bk")
            nc.sync.dma_start(
                bk_sb[:], bk_d.rearrange("h d -> (h d)").rearrange("(o p) -> p o", p=P))
            bvrow = cst.tile([1, C], f32r, tag="bvrow")
            nc.sync.dma_start(bvrow[:], bv_d.rearrange("h d -> (h d)")[None])
            bp_sb = cst.tile([P, CC], f32, tag="bp")
            nc.sync.dma_start(bp_sb[:], bp_d.rearrange("(o p) -> p o", p=P))
            b1f_sb = cst.tile([P, FC], f32, tag="b1f")
            nc.sync.dma_start(b1f_sb[:], b1_d.rearrange("(o p) -> p o", p=P))
            b2_sb = cst.tile([P, CC], f32, tag="b2")
            nc.sync.dma_start(b2_sb[:], b2_d.rearrange("(o p) -> p o", p=P))

            ident = cst.tile([P, P], f32, tag="ident")
            make_identity(nc, ident[:])
            identr_t = cst.tile([P, P], f32r, tag="identr")
            with nc.allow_low_precision(reason="f32r identity"):
                nc.vector.tensor_scalar(identr_t[:], ident[:], 1.0, None, OP.mult)
            identr = identr_t[:]

            ones64 = cst.tile([P, P], f32r, tag="ones64")
            nc.vector.tensor_scalar(ones64[64:65, :], ident[64:65, :], 0.0, 1.0,
                                    OP.mult, OP.add)

            ones_col = cst.tile([1, P], f32r, tag="ones_col")
            nc.vector.tensor_scalar(ones_col[:], ident[0:1, :], 0.0, 1.0,
                                    OP.mult, OP.add)

            # causal masks, multiplicative; each [P, 256]
            # mask_sc[p, tl] = 1 if (p + 128*sc) <= tl else 0
            masks = []
            for sc in range(2):
                mk = cst.tile([P, 256], f32, tag=f"mask{sc}")
                nc.gpsimd.memset(mk[:], 1.0)
                nc.gpsimd.affine_select(
                    out=mk[:], in_=mk[:],
                    compare_op=OP.is_ge, fill=0.0,
                    base=-128 * sc, pattern=[[1, 256]], channel_multiplier=-1)
                masks.append(mk)

            # ---------- per-pair loop (software-pipelined) ----------
            rep_ctx = tc.For_i(0, repeat, 1) if repeat > 1 else contextlib.nullcontext()
            with tc.tile_pool(name="p2", bufs=2) as p2, \
                 tc.tile_pool(name="p1", bufs=1) as p1, \
                 rep_ctx:

                  def stage_dma(pr):
                      x_view = x_d[2 * pr:2 * pr + 2].rearrange(
                          "b (o p) c -> p (b o) c", p=P)
                      x_tok = p2.tile([P, 4, C], f32, tag="x_tok")
                      nc.sync.dma_start(x_tok[:, 0:2], x_view[:, 0:2])
                      nc.scalar.dma_start(x_tok[:, 2:3], x_view[:, 2:3])
                      nc.gpsimd.dma_start(x_tok[:, 3:4], x_view[:, 3:4])
                      return x_tok

                  def layernorm_tokens(src, g_sb, lb_sb, dstT, tagp,
                                       act_copy=True, s_pre=None):
                      """src: [P,4,C] token-major f32. Writes dstT [P,CC,TP]
                      f32r feature-major, gain/bias folded into the copy.
                      s_pre: optional precomputed per-slot row sums [P,4]."""
                      if s_pre is None:
                          # first moment on ACT (accum_out): runs concurrently
                          # with the DVE second-moment passes below
                          s = p1.tile([P, 4], f32, tag=f"{tagp}_s")
                          sdum = p1.tile([P, C], mybir.dt.bfloat16, tag="sdum")
                          for so in range(4):
                              nc.scalar.activation(
                                  sdum[:], src[:, so], AF.Identity,
                                  accum_out=s[:, so:so + 1])
                      else:
                          s = s_pre
                      sq = p1.tile([P, C], f32, tag="sq_scratch")
                      s2 = p1.tile([P, 4], f32, tag=f"{tagp}_s2")
                      for so in range(4):
                          nc.vector.scalar_tensor_tensor(
                              sq[:], src[:, so], 1.0, src[:, so],
                              OP.mult, OP.mult, accum_out=s2[:, so:so + 1])
                      mu = p1.tile([P, 4], f32, tag=f"{tagp}_mu")
                      nc.vector.tensor_scalar_mul(mu[:], s[:], 1.0 / C)
                      var = p1.tile([P, 4], f32, tag=f"{tagp}_var")
                      nc.vector.tensor_scalar(var[:], s2[:], 1.0 / C, EPS,
                                              OP.mult, OP.add)
                      mu2 = p1.tile([P, 4], f32, tag=f"{tagp}_mu2")
                      nc.vector.tensor_tensor(mu2[:], mu[:], mu[:], OP.mult)
                      nc.vector.tensor_tensor(var[:], var[:], mu2[:], OP.subtract)
                      # rstd via bit-trick + Newton (no ACT table switch)
                      yt = p1.tile([P, 4], f32, tag=f"{tagp}_y")
                      ti = p1.tile([P, 4], i32, tag=f"{tagp}_ti")
                      nc.vector.tensor_scalar(ti[:], var[:].bitcast(i32), 1, None,
                                              OP.arith_shift_right)
                      nc.vector.tensor_scalar(yt[:].bitcast(i32), ti[:], -1, MAGIC,
                                              OP.mult, OP.add)
                      y2 = p1.tile([P, 4], f32, tag=f"{tagp}_y2")
                      w = p1.tile([P, 4], f32, tag=f"{tagp}_w")
                      for _ in range(2):
                          nc.vector.tensor_tensor(y2[:], yt[:], yt[:], OP.mult)
                          nc.vector.tensor_tensor(y2[:], y2[:], var[:], OP.mult)
                          nc.vector.tensor_scalar(w[:], y2[:], -0.5, 1.5,
                                                  OP.mult, OP.add)
                          nc.vector.tensor_tensor(yt[:], yt[:], w[:], OP.mult)
                      murs = p1.tile([P, 4], f32, tag=f"{tagp}_murs")
                      nc.vector.tensor_tensor(murs[:], mu[:], yt[:], OP.mult)
                      htok = p1.tile([P, 4, C], f32r, tag="htok", name="htok")
                      with nc.allow_low_precision(reason="f32r htok"):
                          for so in range(4):
                              nc.vector.tensor_scalar(
                                  htok[:, so], src[:, so],
                                  yt[:, so:so + 1], murs[:, so:so + 1],
                                  OP.mult, OP.subtract)
                      for c in range(CC):
                          tp = psum_t()
                          for so in range(4):
                              nc.tensor.transpose(
                                  tp[:, P * so:P * so + P],
                                  htok[:, so, P * c:P * c + P], identr)
                          if act_copy:
                              nc.scalar.activation(
                                  dstT[:, c], tp[:].bitcast(f32), AF.Identity,
                                  bias=lb_sb[:, c:c + 1], scale=g_sb[:, c:c + 1])
                          else:
                              nc.vector.tensor_scalar(
                                  dstT[:, c], tp[:].bitcast(f32),
                                  g_sb[:, c:c + 1], lb_sb[:, c:c + 1],
                                  OP.mult, OP.add)

                  def stage_qkv(x_tok):
                      h1T = p2.tile([P, CC, TP], f32r, tag="h1T")
                      layernorm_tokens(x_tok, g1_sb, lb1_sb, h1T, "ln1")
                      QT = p2.tile([P, CC, TP], f32r, tag="QT")
                      KT = p2.tile([P, CC, TP], f32r, tag="KT")
                      for (W_sb, b_sb, dst) in ((Wq_sb, bq_sb, QT),
                                                (Wk_sb, bk_sb, KT)):
                          for mo in range(CC):
                              ps = psum()
                              for c in range(CC):
                                  nc.tensor.matmul(
                                      ps[:], W_sb[:, c, P * mo:P * mo + P],
                                      h1T[:, c],
                                      start=(c == 0), stop=(c == CC - 1))
                              nc.scalar.activation(dst[:, mo], ps[:], AF.Identity,
                                                   bias=b_sb[:, mo:mo + 1])
                      # V-hat [P, 4, H, 65]: col 64 = 1.0 (denominator row)
                      V_sb = p2.tile([P, 4, H, 65], f32r, tag="V_sb")
                      with nc.allow_low_precision(reason="ones col"):
                          nc.vector.tensor_scalar(
                              V_sb[:, :, :, 64:65],
                              ident[:, 0:1][:, None, None, :]
                              .to_broadcast((P, 4, H, 1)),
                              0.0, 1.0, OP.mult, OP.add)
                      for to in range(4):
                          ps = psum()
                          for c in range(CC):
                              nc.tensor.matmul(
                                  ps[:, 0:C], h1T[:, c, P * to:P * to + P],
                                  Wv_sb[:, c],
                                  start=(c == 0), stop=False)
                          nc.tensor.matmul(ps[:, 0:C], ones_col[:], bvrow[:],
                                           start=False, stop=True)
                          nc.scalar.activation(
                              V_sb[:, to, :, 0:64],
                              ps[:, 0:C].rearrange("p (h d) -> p h d", h=H),
                              AF.Copy)
                      return QT, KT, V_sb

                  x_cur = stage_dma(0)
                  qkv_cur = stage_qkv(x_cur)
                  for pr in range(n_pairs):
                   x_tok = x_cur
                   QT, KT, V_sb = qkv_cur
                   y_view = y_d[2 * pr:2 * pr + 2].rearrange(
                       "b (o p) c -> p (b o) c", p=P)
                   if pr + 1 < n_pairs:
                       x_cur = stage_dma(pr + 1)

                   # ---- attention ----
                   AVT = p1.tile([P, CC, TP], f32r, tag="AVT", name="AVT")
                   for h in range(H):
                       mo, half = h // 2, h % 2
                       rows = slice(64 * half, 64 * half + 64)
                       Es = []
                       tri = masks[0][:, 0:128]
                       for sc in range(2):
                           sps = psum_sc()
                           for bb in range(2):
                               cols = slice(256 * bb, 256 * bb + 256)
                               nc.tensor.matmul(
                                   sps[:, cols],
                                   QT[rows, mo, 256 * bb + 128 * sc:
                                      256 * bb + 128 * sc + 128],
                                   KT[rows, mo, cols],
                                   start=True, stop=True)
                           e = p2.tile([P, TP], f32r, tag=f"E{sc}")
                           ev = e[:].rearrange("p (b t) -> p b t", b=2)
                           evf = e[:].bitcast(f32).rearrange(
                               "p (b t) -> p b t", b=2)
                           spsv = sps[:].rearrange("p (b t) -> p b t", b=2)
                           if sc == 0:
                               # lower t-halves need the triangle; upper
                               # halves are all-keep (p < 128 <= t)
                               nc.scalar.activation(e[:], sps[:], AF.Exp,
                                                    scale=SCALE)
                               nc.gpsimd.tensor_tensor(
                                   ev[:, :, 0:128], evf[:, :, 0:128],
                                   tri[:, None, :].to_broadcast((P, 2, P)),
                                   OP.mult)
                           else:
                               # lower t-halves are fully masked: write 0,
                               # exp only the upper halves + triangle
                               with nc.allow_low_precision(reason="zero f32r"):
                                   nc.gpsimd.tensor_scalar(
                                       ev[:, :, 0:128],
                                       tri[:, None, :].to_broadcast((P, 2, P)),
                                       0.0, None, OP.mult)
                               nc.scalar.activation(
                                   ev[:, :, 128:256], spsv[:, :, 128:256],
                                   AF.Exp, scale=SCALE)
                               nc.gpsimd.tensor_tensor(
                                   ev[:, :, 128:256], evf[:, :, 128:256],
                                   tri[:, None, :].to_broadcast((P, 2, P)),
                                   OP.mult)
                           Es.append(e)
                       avps = psum_av()
                       for bb in range(2):
                           cols = slice(256 * bb, 256 * bb + 256)
                           for sc in range(2):
                               nc.tensor.matmul(
                                   avps[0:65, cols], V_sb[:, 2 * bb + sc, h, :],
                                   Es[sc][:, cols], start=(sc == 0),
                                   stop=(sc == 1))
                       R = p2.tile([P, TP], f32r, tag="R", name="R")
                       with nc.allow_low_precision(reason="softmax denom recip"):
                           nc.vector.reciprocal(R[64:65, :], avps[64:65, :])
                       rps = psum_rp()
                       nc.tensor.matmul(rps[:], ones64[64:65, :], R[64:65, :],
                                        start=True, stop=True)
                       nc.scalar.activation(AVT[rows, mo, :], avps[0:64, :],
                                            AF.Copy)
                       nc.vector.tensor_tensor(AVT[rows, mo],
                                               AVT[rows, mo].bitcast(f32),
                                               rps[rows, :], OP.mult)

                   # ---- proj + residual ----
                   proj_sb = p1.tile([P, CC, TP], f32r, tag="proj_sb")
                   for mo in range(CC):
                       ps = psum()
                       for c in range(CC):
                           nc.tensor.matmul(
                               ps[:], Wp_sb[:, c, P * mo:P * mo + P], AVT[:, c],
                               start=(c == 0), stop=(c == CC - 1))
                       nc.scalar.activation(proj_sb[:, mo], ps[:], AF.Identity,
                                            bias=bp_sb[:, mo:mo + 1])
                   out1_tok = p1.tile([P, 4, C], f32, tag="out1_tok")
                   ln2_s = p1.tile([P, 4], f32, tag="ln2_s")
                   for so in range(4):
                       tp = psum_t()
                       for mo in range(CC):
                           nc.tensor.transpose(
                               tp[:, P * mo:P * mo + P],
                               proj_sb[:, mo, P * so:P * so + P], identr)
                       nc.vector.scalar_tensor_tensor(
                           out1_tok[:, so], tp[:, 0:C].bitcast(f32), 1.0,
                           x_tok[:, so], OP.mult, OP.add,
                           accum_out=ln2_s[:, so:so + 1])

                   # ---- next pair's LN1 + QKV (fills PE during LN2) ----
                   if pr + 1 < n_pairs:
                       qkv_cur = stage_qkv(x_cur)

                   # ---- LN2 + FFN ----
                   h2T = p1.tile([P, CC, TP], f32r, tag="h2T")
                   layernorm_tokens(out1_tok, g2_sb, lb2_sb, h2T, "ln2", s_pre=ln2_s)

                   FF_sb = p1.tile([P, FC, TP], f32r, tag="FF_sb")
                   for fo in range(FC):
                       ps = psum()
                       for c in range(CC):
                           nc.tensor.matmul(
                               ps[:], W1_sb[:, c, P * fo:P * fo + P], h2T[:, c],
                               start=(c == 0), stop=(c == CC - 1))
                       nc.scalar.activation(FF_sb[:, fo], ps[:], AF.Relu,
                                            bias=b1f_sb[:, fo:fo + 1])
                   g_sb = p1.tile([P, CC, TP], f32r, tag="g_sb")
                   for mo in range(CC):
                       ps = psum()
                       for fo in range(FC):
                           nc.tensor.matmul(
                               ps[:], W2_sb[:, fo, P * mo:P * mo + P], FF_sb[:, fo],
                               start=(fo == 0), stop=(fo == FC - 1))
                       nc.scalar.activation(g_sb[:, mo], ps[:], AF.Identity,
                                            bias=b2_sb[:, mo:mo + 1])

                   y_tok = p2.tile([P, 4, C], f32, tag="y_tok")
                   for so in range(4):
                       tp = psum_t()
                       for mo in range(CC):
                           nc.tensor.transpose(
                               tp[:, P * mo:P * mo + P],
                               g_sb[:, mo, P * so:P * so + P], identr)
                       nc.vector.tensor_tensor(y_tok[:, so],
                                               tp[:, 0:C].bitcast(f32),
                                               out1_tok[:, so], OP.add)
                   nc.sync.dma_start(y_view, y_tok[:])

    nc.compile()
    return nc


_NC_CACHE = {}


def kernel(_run_kwargs=None, **inputs) -> np.ndarray:
    run_kwargs = _run_kwargs or {}
    x = np.ascontiguousarray(np.asarray(inputs["x"], dtype=np.float32))
    weights = {k: np.ascontiguousarray(np.asarray(v, dtype=np.float32))
               for k, v in inputs.items() if k != "x"}

    if "nc" not in _NC_CACHE:
        _NC_CACHE["nc"] = build_nc()
    nc = _NC_CACHE["nc"]

    in_maps = []
    for c in range(N_CORES):
        m = {"x": x[c * B_LOCAL:(c + 1) * B_LOCAL]}
        m.update(weights)
        in_maps.append(m)

    res = run_bass_kernel_spmd(nc, in_maps, core_ids=list(range(N_CORES)), **run_kwargs)
    y = np.concatenate([r["y"] for r in res.results], axis=0)
    kernel.last_result = res
    return y
